# revision 36
# baseline (speedup 1.0000x reference)
"""BiMPM forward on 8 Trainium2 NeuronCores (Bass/Tile).

Sharding: 8 cores = (batch b in 0..3) x (side in {p, h}).
  core 2b+0: A = left[b],  B = right[b]   -> mv_p features + agg over mv_p
  core 2b+1: A = right[b], B = left[b]    -> mv_h features + agg over mv_h
Every core runs the same program (SPMD) on its own (A, B) pair:
  ctx BiLSTM over A and B (fw group + bw group, 2 seqs batched per group),
  matching (62 A-side features), agg BiLSTM over mv_A (final hidden states
  only). A tiny second launch computes the final FC from the gathered
  per-core agg states.

Scan structure (latency-optimized):
  The full gate pre-activation (Whh@h + Wih@x_t + b) accumulates in one
  PSUM bank per step: the x-part (+bias via a baked ones-row) is issued
  early so it runs in PE idle time; only the 16 h-matmuls sit on the
  critical path. tanh(g) is folded into a single sigmoid activation over
  all 8 gate chunks via tanh(x) = 2*sigmoid(2x)-1 (g-gate weight rows are
  pre-scaled by 2 on the host), and the c-update uses two fused
  scalar_tensor_tensor ops.

Matching is engine-balanced: the max-attentive q-loop is split across
DVE (fused mul+max), Pool/GPSIMD (same op), and Act (product via
per-partition scale) + DVE (bf16 2x max); squares/rsqrt run on Act.
"""
import sys

sys.path.insert(0, '/opt/trn_rl_repo')

import numpy as np
import ml_dtypes

import concourse.bass as bass
import concourse.mybir as mybir
from concourse import tile, masks
from concourse.bass_utils import run_bass_kernel_spmd

F32 = mybir.dt.float32
BF16 = mybir.dt.bfloat16
AF = mybir.ActivationFunctionType
OP = mybir.AluOpType
AX = mybir.AxisListType

EPS = 1e-8
EPS2 = 1e-16
B, S, D, H, L, NCLS = 4, 256, 300, 256, 10, 22
GH = 4 * H  # 1024 gates
NCHUNK = 8  # 1024 / 128
AGG_IN = 62
DX = D + 1    # x rows + ones row (bias)
AGX = AGG_IN + 1
NEG_BIG = -3.0e38

TRACE = False
DEBUG_OUTS = False

# gate chunk order in PSUM columns: i0 i1 f0 f1 o0 o1 g0 g1
# host permutes weight/bias gate blocks accordingly (torch i f g o -> i f o g)
# and scales the g-gate rows by 2 (tanh(x) = 2*sigmoid(2x) - 1).

# max-attentive q-loop routing by q-pair: D = fused mul+max on DVE,
# A / P = products on Act / Pool into 4 accumulator lanes, folded by one
# merged [128, 4S] DVE max per pair.
XACC_PAT = ('D', 'A', 'P', 'A', 'P')


class PatchedTC(tile.TileContext):
    """This walrus build rejects instructions carrying more than MAX_WAITS sync
    waits. Tile freely attaches many (one per outstanding producer proc).
    After scheduling, split the excess onto same-engine NOP carriers placed
    immediately before the overloaded instruction."""


MAX_WAITS = 1


def _split_waits(nc, maxw=None):
    if maxw is None:
        maxw = MAX_WAITS
    for f in nc.m.functions:
        for blk in f.blocks:
            insts = blk.instructions  # live list
            out = []
            for inst in insts:
                si = getattr(inst, 'sync_info', None)
                waits = list(si.on_wait) if si is not None else []
                if len(waits) > maxw:
                    excess = waits[:-maxw]
                    for w0 in range(0, len(excess), maxw):
                        nop = _make_nop(nc, inst.engine)
                        nop.sync_info = mybir.SyncInfo(
                            on_wait=excess[w0:w0 + maxw], on_update=[])
                        out.append(nop)
                    inst.sync_info = mybir.SyncInfo(
                        on_wait=waits[-maxw:], on_update=list(si.on_update))
                out.append(inst)
            if len(out) != len(insts):
                insts.clear()
                insts.extend(out)


def _make_nop(nc, engine):
    bi = nc.engines[engine].nop(nofuse=True)
    inst = bi.ins
    cur = nc.cur_bb.bb.instructions
    assert cur and cur[-1].name == inst.name
    cur.pop()
    return inst


# ----------------------------------------------------------------------------
# launch 1 program
# ----------------------------------------------------------------------------

def build_launch1():
    nc = bass.Bass()

    dr = {}
    dr['xAB'] = nc.dram_tensor('xAB', [DX, S, 2], BF16, kind='ExternalInput')
    for g in range(2):  # 0=fw 1=bw
        dr[f'ctx_WihT_{g}'] = nc.dram_tensor(f'ctx_WihT_{g}', [DX, GH], BF16, kind='ExternalInput')
        dr[f'ctx_WhhT_{g}'] = nc.dram_tensor(f'ctx_WhhT_{g}', [H, GH], BF16, kind='ExternalInput')
        dr[f'agg_WihT_{g}'] = nc.dram_tensor(f'agg_WihT_{g}', [AGX, GH], BF16, kind='ExternalInput')
        dr[f'agg_WhhT_{g}'] = nc.dram_tensor(f'agg_WhhT_{g}', [H, GH], BF16, kind='ExternalInput')
    # w*w, padded to 32 cols per perspective: tile a = [w3 w4 w5 w6], tile b = [w7 w8]
    dr['wsqT_a_f32'] = nc.dram_tensor('wsqT_a_f32', [H, 128], F32, kind='ExternalInput')
    dr['wsqT_b_f32'] = nc.dram_tensor('wsqT_b_f32', [H, 64], F32, kind='ExternalInput')
    dr['wsqT_a_bf16'] = nc.dram_tensor('wsqT_a_bf16', [H, 128], BF16, kind='ExternalInput')
    dr['wsqT_b_bf16'] = nc.dram_tensor('wsqT_b_bf16', [H, 64], BF16, kind='ExternalInput')

    encB_dram = [nc.dram_tensor(f'encB_dram_{g}', [S, H], BF16) for g in range(2)]
    # rows staged for partition-broadcast: [2, S]: 0 rnB_cos, 1 rsumA_recip
    brow_dram = [nc.dram_tensor(f'brow_dram_{g}', [2, S], F32) for g in range(2)]
    # maxpool rnB rows (set g), one-shot broadcast
    browL_dram = [nc.dram_tensor(f'browL_dram_{g}', [L, S], F32) for g in range(2)]

    dr['agg_out'] = nc.dram_tensor('agg_out', [128, 2, 2], F32, kind='ExternalOutput')
    dr['meanA'] = nc.dram_tensor('meanA', [D], F32, kind='ExternalOutput')
    if DEBUG_OUTS:
        dr['enc_dbg'] = nc.dram_tensor('enc_dbg', [2, 128, 2, 2, S + 1], BF16, kind='ExternalOutput')
        dr['mv_dbg'] = nc.dram_tensor('mv_dbg', [AGX, S], BF16, kind='ExternalOutput')
        dr['sig0_dbg'] = nc.dram_tensor('sig0_dbg', [128, 16], F32, kind='ExternalOutput')

    with PatchedTC(nc) as tc:
        _emit_core_program(nc, tc, dr, encB_dram, brow_dram, browL_dram)
    _split_waits(nc)
    return nc


KCTX = [(0, 128), (128, 128), (256, DX - 256)]  # last tile includes ones row


def _emit_core_program(nc, tc, dr, encB_dram, brow_dram, browL_dram):
    with tc.tile_pool(name='persist', bufs=1) as persist:
        # ---------------- identities, weights, inputs ----------------
        id_bf16 = persist.tile([128, 128], BF16, tag='idb', name='idb')
        id_f32 = persist.tile([128, 128], F32, tag='idf', name='idf')
        masks.make_identity(nc, id_bf16[:])
        masks.make_identity(nc, id_f32[:])

        wih, whh = {}, {}
        for g in range(2):
            wih[g] = []
            for (k0, kn) in KCTX:
                t = persist.tile([kn, GH], BF16, tag=f'wih{g}_{k0}', name=f'wih{g}_{k0}')
                nc.sync.dma_start(t[:], dr[f'ctx_WihT_{g}'][k0:k0 + kn, :])
                wih[g].append(t)
            whh[g] = []
            for k in range(2):
                t = persist.tile([128, GH], BF16, tag=f'whh{g}_{k}', name=f'whh{g}_{k}')
                nc.sync.dma_start(t[:], dr[f'ctx_WhhT_{g}'][k * 128:(k + 1) * 128, :])
                whh[g].append(t)

        awih, awhh = {}, {}
        for g in range(2):
            t = persist.tile([AGX, GH], BF16, tag=f'awih{g}', name=f'awih{g}')
            nc.sync.dma_start(t[:], dr[f'agg_WihT_{g}'][:])
            awih[g] = t
            awhh[g] = []
            for k in range(2):
                t = persist.tile([128, GH], BF16, tag=f'awhh{g}_{k}', name=f'awhh{g}_{k}')
                nc.sync.dma_start(t[:], dr[f'agg_WhhT_{g}'][k * 128:(k + 1) * 128, :])
                awhh[g].append(t)

        # wsq_f[ab][k], wsq_b[ab][k]: fp32/bf16 w^2 tiles; ab=0 -> 128 cols, ab=1 -> 64
        wsq_f, wsq_b = {}, {}
        for ab, nch in ((0, 128), (1, 64)):
            wsq_f[ab], wsq_b[ab] = [], []
            abn = 'a' if ab == 0 else 'b'
            for k in range(2):
                t = persist.tile([128, nch], F32, tag=f'wsqf{abn}{k}', name=f'wsqf{abn}{k}')
                nc.sync.dma_start(t[:], dr[f'wsqT_{abn}_f32'][k * 128:(k + 1) * 128, :])
                wsq_f[ab].append(t)
                t = persist.tile([128, nch], BF16, tag=f'wsqb{abn}{k}', name=f'wsqb{abn}{k}')
                nc.sync.dma_start(t[:], dr[f'wsqT_{abn}_bf16'][k * 128:(k + 1) * 128, :])
                wsq_b[ab].append(t)

        ones_col = persist.tile([128, 1], F32, tag='ones', name='ones')
        nc.vector.memset(ones_col[:], 1.0)
        epsb = persist.tile([128, 1], F32, tag='epsb', name='epsb')
        nc.vector.memset(epsb[:], EPS2)

        # x tiles [kn, S, 2] bf16 (A, B stacked; ones row baked in)
        xk = []
        for (k0, kn) in KCTX:
            t = persist.tile([kn, S, 2], BF16, tag=f'x{k0}', name=f'x{k0}')
            nc.sync.dma_start(t[:], dr['xAB'][k0:k0 + kn, :, :])
            xk.append(t)

        # meanA from bf16 A columns (f32 accumulate)
        macc = persist.tile([128, 3], F32, tag='macc', name='macc')
        msc = persist.tile([128, 3], F32, tag='msc', name='msc')
        nc.vector.memset(macc[:], 0.0)
        for ki, (k0, kn) in enumerate(KCTX):
            rn = min(kn, D - k0)
            nc.vector.tensor_reduce(macc[0:rn, ki:ki + 1], xk[ki][0:rn, :, 0], axis=AX.X, op=OP.add)
        nc.scalar.activation(msc[:], macc[:], AF.Copy, scale=1.0 / S)
        for ki, (k0, kn) in enumerate(KCTX):
            rn = min(kn, D - k0)
            nc.sync.dma_start(dr['meanA'][k0:k0 + rn], msc[0:rn, ki:ki + 1])

        # ---------------- ctx scans ----------------
        # encT[g]: [128, (seq 2, half 2, col S+1)] bf16; fw: h_t at col t+1 (zero col 0),
        # bw: h_t at col t (zero col S).
        encT = {g: persist.tile([128, 2, 2, S + 1], BF16, tag=f'enc{g}', name=f'enc{g}') for g in range(2)}
        for g in range(2):
            zc = 0 if g == 0 else S
            nc.vector.memset(encT[g][:, :, :, zc], 0.0)

        def ctx_x(g, t, ps):
            # start=True only on the FIRST matmul into the bank: start marks
            # the whole 2KB zero-region pending-zero; repeating it would wipe
            # earlier chunks' accumulation.
            for c in range(NCHUNK):
                for ki in range(3):
                    nc.tensor.matmul(
                        ps[:, c * 2:(c + 1) * 2],
                        wih[g][ki][:, c * 128:(c + 1) * 128], xk[ki][:, t, :],
                        start=(c == 0 and ki == 0), stop=False, skip_group_check=True)

        _emit_scan(nc, tc, 'ctx', whh, ctx_x, encT, M=2, final_out=None,
                   dbg=dr.get('sig0_dbg'))
        if DEBUG_OUTS:
            for g in range(2):
                nc.sync.dma_start(dr['enc_dbg'][g], encT[g][:])

        # ---------------- matching ----------------
        mv63 = persist.tile([AGX, S], BF16, tag='mv63', name='mv63')
        ones_row = persist.tile([1, S], BF16, tag='onesrow', name='onesrow')
        nc.vector.memset(ones_row[:], 1.0)
        nc.sync.dma_start(mv63[AGG_IN:AGX, :], ones_row[:])  # ones row (agg bias)
        _emit_matching(nc, tc, dr, encT, encB_dram, brow_dram, browL_dram,
                       wsq_f, wsq_b, ones_col, epsb, id_bf16, id_f32, mv63)

        if DEBUG_OUTS:
            nc.sync.dma_start(dr['mv_dbg'][:], mv63[:])

        # ---------------- agg ----------------
        aencT = {g: persist.tile([128, 1, 2, S + 1], BF16, tag=f'aenc{g}', name=f'aenc{g}') for g in range(2)}
        for g in range(2):
            zc = 0 if g == 0 else S
            nc.vector.memset(aencT[g][:, :, :, zc], 0.0)

        def agg_x(g, t, ps):
            for c in range(NCHUNK):
                nc.tensor.matmul(
                    ps[:, c:c + 1], awih[g][:, c * 128:(c + 1) * 128], mv63[:, t:t + 1],
                    start=(c == 0), stop=False, skip_group_check=True)

        final_h = persist.tile([128, 2, 2], F32, tag='finalh', name='finalh')  # (group, half)
        _emit_scan(nc, tc, 'agg', awhh, agg_x, aencT, M=1, final_out=final_h)
        nc.sync.dma_start(dr['agg_out'][:], final_h[:])


def _emit_scan(nc, tc, name, whh, emit_x, encT, M, final_out, dbg=None):
    """Interleaved fw/bw scan groups.
    encT[g]: [128, (M seq, 2 half, S+1)] bf16.
    whh[g]: 2 k-tiles [128, 1024] bf16, gate chunks i0 i1 f0 f1 o0 o1 g0 g1
    (g rows pre-scaled 2x). emit_x(g, t, ps) emits the x+bias part for
    step t into psum tile ps (start=True, stop=False)."""
    with (
        tc.tile_pool(name=f'{name}_ps0', bufs=2, space='PSUM') as pp0,
        tc.tile_pool(name=f'{name}_ps1', bufs=2, space='PSUM') as pp1,
        tc.tile_pool(name=f'{name}_sb', bufs=3) as sb,
        tc.tile_pool(name=f'{name}_cs', bufs=1) as csp,
    ):
        pps = {0: pp0, 1: pp1}
        c_state, banks = {}, {}
        for g in range(2):
            c_state[g] = csp.tile([128, 2 * M], F32, tag=f'c{g}', name=f'c{g}')  # (half, m)
            nc.vector.memset(c_state[g][:], 0.0)
        for step in range(S):
            for g in range(2):
                t = step if g == 0 else S - 1 - step
                rd = t if g == 0 else t + 1
                wr = t + 1 if g == 0 else t
                ps = pps[g].tile([128, NCHUNK * M], F32, tag=f'gsum{g}', name=f'gsum{g}')
                emit_x(g, t, ps)
                n_mm = 0
                for c in range(NCHUNK):
                    for k in range(2):
                        nc.tensor.matmul(
                            ps[:, c * M:(c + 1) * M],
                            whh[g][k][:, c * 128:(c + 1) * 128],
                            encT[g][:, :, k, rd],
                            start=False, stop=(n_mm == 15), skip_group_check=True)
                        n_mm += 1
                # all 8 chunks through one sigmoid; chunks 6,7 hold sigma(2g)
                sig = sb.tile([128, NCHUNK * M], F32, tag=f'sig{g}', name=f'sig{g}')
                nc.scalar.activation(sig[:], ps[:], AF.Sigmoid)
                if dbg is not None and step == 0 and g == 0:
                    nc.sync.dma_start(dbg[:], sig[:])
                cs = c_state[g]
                t2 = sb.tile([128, 2 * M], F32, tag=f't2{g}', name=f't2{g}')
                nc.vector.tensor_tensor(t2[:], sig[:, 2 * M:4 * M], cs[:], OP.mult)
                t1 = sb.tile([128, 2 * M], F32, tag=f't1{g}', name=f't1{g}')
                nc.vector.scalar_tensor_tensor(
                    t1[:], sig[:, 6 * M:8 * M], -0.5, sig[:, 0:2 * M],
                    op0=OP.add, op1=OP.mult)
                # c = 2*t1 + t2
                nc.vector.scalar_tensor_tensor(
                    cs[:], t1[:], 2.0, t2[:], op0=OP.mult, op1=OP.add)
                th = sb.tile([128, 2 * M], F32, tag=f'th{g}', name=f'th{g}')
                nc.scalar.activation(th[:], cs[:], AF.Tanh)
                # h = sig_o * th; encT dest dims (m, h) permuted to (h, m)
                hout = encT[g][:, :, :, wr].transpose([0, 2, 1])
                nc.vector.tensor_tensor(
                    hout, sig[:, 4 * M:6 * M].rearrange('p (h m) -> p h m', h=2),
                    th[:].rearrange('p (h m) -> p h m', h=2), OP.mult)
                if final_out is not None and step == S - 1:
                    nc.vector.tensor_tensor(final_out[:, g, :], sig[:, 4 * M:6 * M],
                                            th[:], OP.mult)


def _emit_matching(nc, tc, dr, encT, encB_dram, brow_dram, browL_dram,
                   wsq_f, wsq_b, ones_col, epsb, id_bf16, id_f32, mv63):
    """A-side matching features into mv63 rows 0:62 bf16.

    rows: 0 cos-max, 1 cos-mean, 2:12 maxpool-f(w3), 12:22 maxpool-b(w4),
          22:32 attentive-f(w5), 32:42 attentive-b(w6),
          42:52 max-attentive-f(w7), 52:62 max-attentive-b(w8)
    w-set s in 0..5 lives in wsq tile s//4 at col offset 32*(s%4), 10 cols wide.
    Phases are emitted interleaved between g=0 and g=1 so the in-order
    engine queues always have independent work from the other direction.
    """
    QB = 16
    with tc.tile_pool(name='m_sb', bufs=1) as msb:
        colfeat = msb.tile([128, 2, 22], F32, tag='colfeat', name='colfeat')
        st = {g: {} for g in range(2)}
        for g in range(2):
            c0 = 1 if g == 0 else 0
            st[g]['eAT'] = [encT[g][:, 0, k, c0:c0 + S] for k in range(2)]
            st[g]['eBT'] = [encT[g][:, 1, k, c0:c0 + S] for k in range(2)]

        def phaseA(g):
            s = st[g]
            eAT, eBT = s['eAT'], s['eBT']
            # squares (Act)
            sqA = [msb.tile([128, S], F32, tag=f'sqA{g}{k}', name=f'sqA{g}{k}') for k in range(2)]
            sqB = [msb.tile([128, S], F32, tag=f'sqB{g}{k}', name=f'sqB{g}{k}') for k in range(2)]
            for k in range(2):
                nc.scalar.activation(sqA[k][:], eAT[k], AF.Square)
                nc.scalar.activation(sqB[k][:], eBT[k], AF.Square)
            s['sqA'], s['sqB'] = sqA, sqB
            # cos recip norms: 1/sqrt(sum sq + eps^2)
            rnA = msb.tile([128, 2], F32, tag=f'rnA{g}', name=f'rnA{g}')
            rnB = msb.tile([128, 2], F32, tag=f'rnB{g}', name=f'rnB{g}')
            with tc.tile_pool(name=f'mn{g}', bufs=2, space='PSUM') as mps:
                for dst, sq in ((rnA, sqA), (rnB, sqB)):
                    ps = mps.tile([128, 2], F32, tag='nsq', name='nsq')
                    for pt in range(2):
                        for k in range(2):
                            nc.tensor.matmul(ps[:, pt:pt + 1],
                                             sq[k][:, pt * 128:(pt + 1) * 128],
                                             ones_col[:], start=(k == 0), stop=(k == 1),
                                             skip_group_check=True)
                    nc.scalar.activation(dst[:], ps[:], AF.Sqrt, bias=epsb[:])
                    nc.vector.reciprocal(dst[:], dst[:])
            for pt in range(2):
                nc.sync.dma_start(brow_dram[g][0, pt * 128:(pt + 1) * 128], rnB[:, pt:pt + 1])
            s['rnA'] = rnA

        def phaseB(g):
            # att + rr + transposes + encB dma
            s = st[g]
            eAT, eBT = s['eAT'], s['eBT']
            att = [msb.tile([128, S], F32, tag=f'att{g}{pt}', name=f'att{g}{pt}') for pt in range(2)]
            rsum = msb.tile([128, 2], F32, tag=f'rsum{g}', name=f'rsum{g}')
            with (
                tc.tile_pool(name=f'ma{g}', bufs=2, space='PSUM') as mps,
                tc.tile_pool(name=f'mab{g}', bufs=1) as bcp,
            ):
                rnB_bc = bcp.tile([128, S], F32, tag='rnBbc', name='rnBbc')
                nc.sync.dma_start(rnB_bc[:], brow_dram[g][0:1, :].partition_broadcast(128)[:, 0, :])
                for pt in range(2):
                    nps = mps.tile([128, S], F32, tag='num', name='num')
                    for k in range(2):
                        nc.tensor.matmul(nps[:], eAT[k][:, pt * 128:(pt + 1) * 128],
                                         eBT[k], start=(k == 0), stop=(k == 1),
                                         skip_group_check=True)
                    nc.vector.scalar_tensor_tensor(
                        att[pt][:], nps[:], s['rnA'][:, pt:pt + 1], rnB_bc[:],
                        op0=OP.mult, op1=OP.mult, accum_out=rsum[:, pt:pt + 1])
                if g == 0:  # cos max/mean features use att_fw only
                    for pt in range(2):
                        nc.vector.tensor_reduce(colfeat[:, pt, 0:1], att[pt][:], axis=AX.X, op=OP.max)
                    nc.scalar.activation(colfeat[:, :, 1], rsum[:], AF.Copy, scale=1.0 / S)
                rr = msb.tile([128, 2], F32, tag=f'rr{g}', name=f'rr{g}')
                nc.vector.tensor_scalar_max(rr[:], rsum[:], EPS)
                nc.vector.reciprocal(rr[:], rr[:])
                for pt in range(2):
                    nc.sync.dma_start(brow_dram[g][1, pt * 128:(pt + 1) * 128], rr[:, pt:pt + 1])
            encB = [msb.tile([128, S], BF16, tag=f'encB{g}{qt}', name=f'encB{g}{qt}') for qt in range(2)]
            attT = [msb.tile([128, S], BF16, tag=f'attT{g}{qt}', name=f'attT{g}{qt}') for qt in range(2)]
            with tc.tile_pool(name=f'mt{g}', bufs=4, space='PSUM') as mps:
                for qt in range(2):
                    for hf in range(2):
                        tp = mps.tile([128, 128], BF16, tag='tpb', name='tpb')
                        nc.tensor.transpose(tp[:], eBT[hf][:, qt * 128:(qt + 1) * 128], id_bf16[:])
                        nc.scalar.copy(encB[qt][:, hf * 128:(hf + 1) * 128], tp[:])
                    for pt in range(2):
                        tpf = mps.tile([128, 128], F32, tag='tpf', name='tpf')
                        nc.tensor.transpose(tpf[:], att[pt][:, qt * 128:(qt + 1) * 128], id_f32[:])
                        nc.scalar.copy(attT[qt][:, pt * 128:(pt + 1) * 128], tpf[:])
                    nc.sync.dma_start(encB_dram[g][qt * 128:(qt + 1) * 128, :], encB[qt][:])
            s['att'], s['encB'], s['attT'] = att, encB, attT

        def phaseC(g):
            # attentive mean + norm sets + maxpool
            s = st[g]
            eAT, eBT = s['eAT'], s['eBT']
            meanT = [msb.tile([128, S], BF16, tag=f'meanT{g}{ht}', name=f'meanT{g}{ht}') for ht in range(2)]
            with (
                tc.tile_pool(name=f'mm{g}', bufs=2, space='PSUM') as mps,
                tc.tile_pool(name=f'mmb{g}', bufs=1) as bcp,
            ):
                rr_bc = bcp.tile([128, S], F32, tag='rrbc', name='rrbc')
                nc.sync.dma_start(rr_bc[:], brow_dram[g][1:2, :].partition_broadcast(128)[:, 0, :])
                for ht in range(2):
                    mp = mps.tile([128, S], F32, tag='meanps', name='meanps')
                    for qt in range(2):
                        nc.tensor.matmul(mp[:], s['encB'][qt][:, ht * 128:(ht + 1) * 128],
                                         s['attT'][qt][:],
                                         start=(qt == 0), stop=(qt == 1), skip_group_check=True)
                    nc.vector.tensor_tensor(meanT[ht][:], mp[:], rr_bc[:], OP.mult)
            s['meanT'] = meanT
            nsA = [msb.tile([128, S], F32, tag=f'nsA{g}{ab}', name=f'nsA{g}{ab}') for ab in range(2)]
            base = 32 * g
            rnA_l10 = msb.tile([128, S], F32, tag=f'rnAl10{g}', name=f'rnAl10{g}')
            rnB_l10 = msb.tile([128, S], F32, tag=f'rnBl10{g}', name=f'rnBl10{g}')
            with tc.tile_pool(name=f'mns{g}', bufs=2, space='PSUM') as mps:
                for ab, nch in ((0, 128), (1, 64)):
                    ps = mps.tile([128, S], F32, tag='nset', name='nset')
                    for k in range(2):
                        nc.tensor.matmul(ps[0:nch, :], wsq_f[ab][k][:], s['sqA'][k][:],
                                         start=(k == 0), stop=(k == 1), skip_group_check=True)
                    nc.scalar.copy(nsA[ab][0:nch, :], ps[0:nch, :])
                    if ab == 0:
                        nc.scalar.activation(rnA_l10[base:base + L, :], ps[base:base + L, :],
                                             AF.Sqrt, bias=epsb[base:base + L, :])
                        nc.vector.reciprocal(rnA_l10[base:base + L, :],
                                             rnA_l10[base:base + L, :])
                        ps2 = mps.tile([128, S], F32, tag='nset', name='nset')
                        for k in range(2):
                            nc.tensor.matmul(ps2[0:nch, :], wsq_f[ab][k][:], s['sqB'][k][:],
                                             start=(k == 0), stop=(k == 1), skip_group_check=True)
                        nc.scalar.activation(rnB_l10[base:base + L, :], ps2[base:base + L, :],
                                             AF.Sqrt, bias=epsb[base:base + L, :])
                        nc.vector.reciprocal(rnB_l10[base:base + L, :],
                                             rnB_l10[base:base + L, :])
                        nc.sync.dma_start(browL_dram[g][:], rnB_l10[base:base + L, :])
            s['nsA'] = nsA
            # maxpool
            rnA_l = msb.tile([128, 2, L], F32, tag=f'rnAl{g}', name=f'rnAl{g}')
            mp_acc = msb.tile([128, 2, L], F32, tag=f'mpacc{g}', name=f'mpacc{g}')
            scr = msb.tile([128, 2, S], F32, tag=f'mpscr{g}', name=f'mpscr{g}')
            with (
                tc.tile_pool(name=f'mp{g}', bufs=3, space='PSUM') as mps,
                tc.tile_pool(name=f'mpb{g}', bufs=1) as bcp,
                tc.tile_pool(name=f'mpw{g}', bufs=2) as wap,
            ):
                rl_all = bcp.tile([128, L, S], F32, tag='rlall', name='rlall')
                nc.sync.dma_start(rl_all[:], browL_dram[g][:, :].partition_broadcast(128))
                for pt in range(2):
                    tpf = mps.tile([128, L], F32, tag='tprn', name='tprn')
                    nc.tensor.transpose(tpf[:], rnA_l10[base:base + L, pt * 128:(pt + 1) * 128],
                                        id_f32[base:base + L, base:base + L])
                    nc.scalar.copy(rnA_l[:, pt, :], tpf[:])
                for l in range(L):
                    wa = [wap.tile([128, S], BF16, tag=f'wa{k}', name=f'wa{k}') for k in range(2)]
                    for k in range(2):
                        nc.scalar.activation(wa[k][:], eAT[k], AF.Copy,
                                             scale=wsq_f[0][k][:, base + l:base + l + 1])
                    for pt in range(2):
                        nps = mps.tile([128, S], F32, tag='mpnum', name='mpnum')
                        for k in range(2):
                            nc.tensor.matmul(nps[:], wa[k][:, pt * 128:(pt + 1) * 128], eBT[k],
                                             start=(k == 0), stop=(k == 1), skip_group_check=True)
                        nc.vector.tensor_tensor(scr[:, pt, :], nps[:], rl_all[:, l, :], OP.mult)
                    nc.vector.tensor_reduce(mp_acc[:, :, l:l + 1], scr[:], axis=AX.X, op=OP.max)
                for pt in range(2):
                    nc.vector.tensor_tensor(colfeat[:, pt, 2 + g * L:2 + (g + 1) * L],
                                            mp_acc[:, pt, :], rnA_l[:, pt, :], OP.mult)

        # max-attentive accumulators + per-block emitters (interleaved g0/g1)
        def xacc_init(g):
            s = st[g]
            s['xaccD'] = [msb.tile([128, S], BF16, tag=f'xaD{g}{pt}', name=f'xaD{g}{pt}')
                          for pt in range(2)]
            s['xaccM'] = msb.tile([128, 2, 2, S], BF16, tag=f'xaM{g}', name=f'xaM{g}')
            for pt in range(2):
                nc.vector.memset(s['xaccD'][pt][:], NEG_BIG)
            nc.vector.memset(s['xaccM'][:], NEG_BIG)

        def xacc_block(g, q0):
            s = st[g]
            att = s['att']
            vb = s['vbp'].tile([128, QB, H], BF16, tag='vbc', name='vbc')
            nc.sync.dma_start(vb[:], encB_dram[g][q0:q0 + QB, :].partition_broadcast(128))
            for qq in range(0, QB, 2):
                q = q0 + qq
                eng = XACC_PAT[(q // 2) % len(XACC_PAT)]
                if eng == 'D':
                    for dq in range(2):
                        for pt in range(2):
                            nc.vector.scalar_tensor_tensor(
                                s['xaccD'][pt][:], vb[:, qq + dq, :],
                                att[pt][:, q + dq:q + dq + 1],
                                s['xaccD'][pt][:], op0=OP.mult, op1=OP.max)
                else:
                    tq = s['tqp'].tile([128, 2, 2, S], BF16, tag='tq', name='tq')
                    for dq in range(2):
                        for pt in range(2):
                            if eng == 'A':
                                nc.scalar.activation(
                                    tq[:, dq, pt, :], vb[:, qq + dq, :], AF.Copy,
                                    scale=att[pt][:, q + dq:q + dq + 1])
                            else:
                                nc.gpsimd.tensor_scalar_mul(
                                    tq[:, dq, pt, :], vb[:, qq + dq, :],
                                    att[pt][:, q + dq:q + dq + 1])
                    nc.vector.tensor_tensor(s['xaccM'][:], tq[:], s['xaccM'][:], OP.max)

        def xacc_fini(g):
            s = st[g]
            xacc = [msb.tile([128, S], BF16, tag=f'xacc{g}{pt}', name=f'xacc{g}{pt}')
                    for pt in range(2)]
            for pt in range(2):
                nc.vector.tensor_tensor(xacc[pt][:], s['xaccM'][:, 0, pt, :],
                                        s['xaccM'][:, 1, pt, :], OP.max)
                nc.vector.tensor_tensor(xacc[pt][:], xacc[pt][:], s['xaccD'][pt][:], OP.max)
            xT_ = [msb.tile([128, S], BF16, tag=f'xT{g}{ht}', name=f'xT{g}{ht}') for ht in range(2)]
            with tc.tile_pool(name=f'mxp{g}', bufs=4, space='PSUM') as mps:
                for ht in range(2):
                    for pt in range(2):
                        tp = mps.tile([128, 128], BF16, tag='tpx', name='tpx')
                        nc.tensor.transpose(tp[:], xacc[pt][:, ht * 128:(ht + 1) * 128], id_bf16[:])
                        nc.scalar.copy(xT_[ht][:, pt * 128:(pt + 1) * 128], tp[:])
            s['xT'] = xT_

        def phaseD(g):
            # final mp_match: (meanT, set 2+g) rows 22+10g; (xT, set 4+g) rows 42+10g
            s = st[g]
            eAT = s['eAT']
            for vT, set_, row0 in ((s['meanT'], 2 + g, 22 + g * L), (s['xT'], 4 + g, 42 + g * L)):
                ab, off = divmod(set_, 4)
                off *= 32
                prod = [msb.tile([128, S], BF16, tag=f'prod{g}{row0}{k}', name=f'prod{g}{row0}{k}')
                        for k in range(2)]
                vsq = [msb.tile([128, S], F32, tag=f'vsq{g}{row0}{k}', name=f'vsq{g}{row0}{k}')
                       for k in range(2)]
                for k in range(2):
                    nc.vector.tensor_tensor(prod[k][:], eAT[k], vT[k][:], OP.mult)
                    nc.scalar.activation(vsq[k][:], vT[k][:], AF.Square)
                # stage this set's A-norms at base partition 0 (engine ops need equal bases)
                n1s = msb.tile([L, S], F32, tag=f'n1s{g}{row0}', name=f'n1s{g}{row0}')
                nc.sync.dma_start(n1s[:], s['nsA'][ab][off:off + L, :])
                feat = msb.tile([L, S], BF16, tag=f'feat{g}{row0}', name=f'feat{g}{row0}')
                with tc.tile_pool(name=f'mf{g}{row0}', bufs=2, space='PSUM') as mps:
                    nump = mps.tile([128, S], F32, tag='nump', name='nump')
                    n2p = mps.tile([128, S], F32, tag='n2p', name='n2p')
                    for k in range(2):
                        nc.tensor.matmul(nump[0:L, :], wsq_b[ab][k][:, off:off + L],
                                         prod[k][:], start=(k == 0), stop=(k == 1),
                                         skip_group_check=True)
                        nc.tensor.matmul(n2p[0:L, :], wsq_f[ab][k][:, off:off + L],
                                         vsq[k][:], start=(k == 0), stop=(k == 1),
                                         skip_group_check=True)
                    den = msb.tile([L, S], F32, tag=f'den{g}{row0}', name=f'den{g}{row0}')
                    nc.vector.tensor_tensor(den[:], n2p[0:L, :], n1s[:], OP.mult)
                    denr = msb.tile([L, S], F32, tag=f'denr{g}{row0}', name=f'denr{g}{row0}')
                    nc.scalar.activation(denr[:], den[:], AF.Sqrt, bias=epsb[0:L, :])
                    nc.vector.reciprocal(denr[:], denr[:])
                    nc.vector.tensor_tensor(feat[:], nump[0:L, :], denr[:], OP.mult)
                nc.sync.dma_start(mv63[row0:row0 + L, :], feat[:])

        # ---- interleaved emission ----
        for g in range(2):
            phaseA(g)
        for g in range(2):
            phaseB(g)
        for g in range(2):
            xacc_init(g)
        for g in range(2):
            phaseC(g)
        with (
            tc.tile_pool(name='mx0', bufs=2) as vbp0,
            tc.tile_pool(name='mxt0', bufs=3) as tqp0,
            tc.tile_pool(name='mx1', bufs=2) as vbp1,
            tc.tile_pool(name='mxt1', bufs=3) as tqp1,
        ):
            st[0]['vbp'], st[0]['tqp'] = vbp0, tqp0
            st[1]['vbp'], st[1]['tqp'] = vbp1, tqp1
            for q0 in range(0, S, QB):
                xacc_block(0, q0)
                xacc_block(1, q0)
        for g in range(2):
            xacc_fini(g)
        for g in range(2):
            phaseD(g)

        # --- transpose column features into mv63 rows 0:22
        with tc.tile_pool(name='cf_ps', bufs=2, space='PSUM') as cps:
            for pt in range(2):
                tp = cps.tile([22, 128], F32, tag='tpcf', name='tpcf')
                nc.tensor.transpose(tp[:], colfeat[:, pt, :], id_f32[:])
                nc.scalar.copy(mv63[0:22, pt * 128:(pt + 1) * 128], tp[:])


def build_launch2():
    nc = bass.Bass()
    NX = 4 * H + 2 + 2 * D  # 1626
    NT = 13                 # padded to 13 * 128 rows
    NH = 2 * H  # 512
    xT = nc.dram_tensor('xT', [128, NT, B], BF16, kind='ExternalInput')
    w1T = nc.dram_tensor('w1T', [128, NT, NH], BF16, kind='ExternalInput')
    b1 = nc.dram_tensor('b1', [128, 4], F32, kind='ExternalInput')
    w2T = nc.dram_tensor('w2T', [128, 4, NCLS], F32, kind='ExternalInput')
    b2 = nc.dram_tensor('b2', [NCLS, 1], F32, kind='ExternalInput')
    yT = nc.dram_tensor('yT', [NCLS, B], F32, kind='ExternalOutput')

    with PatchedTC(nc) as tc:
        with (
            tc.tile_pool(name='sb', bufs=1) as sb,
            tc.tile_pool(name='ps', bufs=4, space='PSUM') as pp,
        ):
            xt = sb.tile([128, NT, B], BF16, tag='x', name='x')
            nc.sync.dma_start(xt[:], xT[:])
            w1 = sb.tile([128, NT, NH], BF16, tag='w1', name='w1')
            nc.sync.dma_start(w1[:], w1T[:])
            b1t = sb.tile([128, 4], F32, tag='b1', name='b1')
            nc.sync.dma_start(b1t[:], b1[:])
            w2 = sb.tile([128, 4, NCLS], F32, tag='w2', name='w2')
            nc.sync.dma_start(w2[:], w2T[:])
            b2t = sb.tile([NCLS, 1], F32, tag='b2', name='b2')
            nc.sync.dma_start(b2t[:], b2[:])
            hT = sb.tile([128, 4, B], F32, tag='hT', name='hT')
            for c in range(4):
                ps = pp.tile([128, B], F32, tag='h', name='h')
                for i in range(NT):
                    nc.tensor.matmul(ps[:], w1[:, i, c * 128:(c + 1) * 128], xt[:, i, :],
                                     start=(i == 0), stop=(i == NT - 1),
                                     skip_group_check=True)
                nc.scalar.activation(hT[:, c, :], ps[:], AF.Tanh, bias=b1t[:, c:c + 1])
            ps = pp.tile([NCLS, B], F32, tag='y', name='y')
            for c in range(4):
                nc.tensor.matmul(ps[:], w2[:, c, :], hT[:, c, :],
                                 start=(c == 0), stop=(c == 3), skip_group_check=True)
            yt = sb.tile([NCLS, B], F32, tag='yt', name='yt')
            nc.scalar.activation(yt[:], ps[:], AF.Identity, bias=b2t[:])
            nc.sync.dma_start(yT[:], yt[:])
    _split_waits(nc)
    return nc


# ----------------------------------------------------------------------------
# host orchestration
# ----------------------------------------------------------------------------

_cache = {}


def _gate_perm():
    # torch gate order (i, f, g, o) blocks of H -> chip order (i, f, o, g),
    # and within each gate the two 128-halves stay in order.
    idx = np.arange(GH).reshape(4, H)
    return np.concatenate([idx[0], idx[1], idx[3], idx[2]])


def _prep_host(inputs):
    bf = ml_dtypes.bfloat16
    perm = _gate_perm()
    pr = {}
    for g, d in ((0, 'f'), (1, 'b')):
        for pref, nin in (('ctx', D), ('agg', AGG_IN)):
            wih = np.asarray(inputs[f'{pref}_Wih_{d}'], np.float32)[perm]  # [1024, IN]
            whh = np.asarray(inputs[f'{pref}_Whh_{d}'], np.float32)[perm]
            bb = np.asarray(inputs[f'{pref}_b_{d}'], np.float32)[perm]
            # tanh(g) = 2*sigmoid(2g) - 1: scale g-gate rows (chip rows 3H:4H) by 2
            wih[3 * H:] *= 2.0
            whh[3 * H:] *= 2.0
            bb = bb.copy()
            bb[3 * H:] *= 2.0
            wihx = np.concatenate([wih.T, bb[None, :]], axis=0)  # [IN+1, GH]
            pr[f'{pref}_WihT_{g}'] = np.ascontiguousarray(wihx).astype(bf)
            pr[f'{pref}_WhhT_{g}'] = np.ascontiguousarray(whh.T).astype(bf)
    # padded w^2 sets: 32 rows per perspective; tile a = w3..w6, tile b = w7, w8
    wsq_pad = np.zeros((6 * 32, H), np.float32)
    for i in range(6):
        wsq_pad[i * 32:i * 32 + L] = np.asarray(inputs[f'mp_w{i + 3}'], np.float32) ** 2
    pr['wsqT_a_f32'] = np.ascontiguousarray(wsq_pad[0:128].T)
    pr['wsqT_b_f32'] = np.ascontiguousarray(wsq_pad[128:192].T)
    pr['wsqT_a_bf16'] = pr['wsqT_a_f32'].astype(bf)
    pr['wsqT_b_bf16'] = pr['wsqT_b_f32'].astype(bf)
    return pr


def kernel(**inputs):
    if 'l1' not in _cache:
        _cache['l1'] = build_launch1()
        _cache['l2'] = build_launch2()
    nc1, nc2 = _cache['l1'], _cache['l2']

    bf = ml_dtypes.bfloat16
    pr = _prep_host(inputs)
    left = np.asarray(inputs['left'], np.float32)
    right = np.asarray(inputs['right'], np.float32)

    in_maps = []
    for b in range(B):
        for side in range(2):
            A = left[b] if side == 0 else right[b]
            Bx = right[b] if side == 0 else left[b]
            m = dict(pr)
            xab = np.empty((DX, S, 2), np.float32)
            xab[0:D, :, 0] = A.T
            xab[0:D, :, 1] = Bx.T
            xab[D, :, :] = 1.0
            m['xAB'] = xab.astype(bf)
            in_maps.append(m)

    res1 = run_bass_kernel_spmd(nc1, in_maps, list(range(8)), trace=TRACE)

    # assemble x [4, 1626]
    xs = []
    for b in range(B):
        rp = res1.results[2 * b]
        rh = res1.results[2 * b + 1]
        ap_f = rp['agg_out'][:, 0, :].T.reshape(-1)
        ap_b = rp['agg_out'][:, 1, :].T.reshape(-1)
        ah_f = rh['agg_out'][:, 0, :].T.reshape(-1)
        ah_b = rh['agg_out'][:, 1, :].T.reshape(-1)
        meanL = rp['meanA']
        meanR = rh['meanA']
        xs.append(np.concatenate([ap_f, ap_b, ah_f, ah_b, [0.5, 0.5], meanL, meanR]))
    x = np.stack(xs).astype(np.float32)

    NX, NT, NH = 4 * H + 2 + 2 * D, 13, 2 * H
    xp = np.zeros((NT * 128, B), np.float32)
    xp[0:NX] = x.T
    w1p = np.zeros((NT * 128, NH), np.float32)
    w1p[0:NX] = np.asarray(inputs['fc1_W'], np.float32).T
    m2 = {
        'xT': xp.reshape(128, NT, B, order='F').copy() if False else
              np.ascontiguousarray(xp.reshape(NT, 128, B).transpose(1, 0, 2)).astype(bf),
        'w1T': np.ascontiguousarray(w1p.reshape(NT, 128, NH).transpose(1, 0, 2)).astype(bf),
        'b1': np.ascontiguousarray(np.asarray(inputs['fc1_b'], np.float32).reshape(4, 128).T),
        'w2T': np.ascontiguousarray(
            np.asarray(inputs['fc2_W'], np.float32).T.reshape(4, 128, NCLS).transpose(1, 0, 2)),
        'b2': np.asarray(inputs['fc2_b'], np.float32).reshape(NCLS, 1),
    }
    res2 = run_bass_kernel_spmd(nc2, [m2], [0])
    y = res2.results[0]['yT'].T
    _cache['last_exec_ns'] = (res1.exec_time_ns, res2.exec_time_ns)
    return np.ascontiguousarray(y.astype(np.float32))


# revision 37
# speedup vs baseline: 1.0035x; 1.0035x over previous
"""BiMPM forward on 8 Trainium2 NeuronCores (Bass/Tile).

Sharding: 8 cores = (batch b in 0..3) x (side in {p, h}).
  core 2b+0: A = left[b],  B = right[b]   -> mv_p features + agg over mv_p
  core 2b+1: A = right[b], B = left[b]    -> mv_h features + agg over mv_h
Every core runs the same program (SPMD) on its own (A, B) pair:
  ctx BiLSTM over A and B (fw group + bw group, 2 seqs batched per group),
  matching (62 A-side features), agg BiLSTM over mv_A (final hidden states
  only). A tiny second launch computes the final FC from the gathered
  per-core agg states.

Scan structure (latency-optimized):
  The full gate pre-activation (Whh@h + Wih@x_t + b) accumulates in one
  PSUM bank per step: the x-part (+bias via a baked ones-row) is issued
  early so it runs in PE idle time; only the 16 h-matmuls sit on the
  critical path. tanh(g) is folded into a single sigmoid activation over
  all 8 gate chunks via tanh(x) = 2*sigmoid(2x)-1 (g-gate weight rows are
  pre-scaled by 2 on the host), and the c-update uses two fused
  scalar_tensor_tensor ops.

Matching is engine-balanced: the max-attentive q-loop is split across
DVE (fused mul+max), Pool/GPSIMD (same op), and Act (product via
per-partition scale) + DVE (bf16 2x max); squares/rsqrt run on Act.
"""
import sys

sys.path.insert(0, '/opt/trn_rl_repo')

import numpy as np
import ml_dtypes

import concourse.bass as bass
import concourse.mybir as mybir
from concourse import tile, masks
from concourse.bass_utils import run_bass_kernel_spmd

F32 = mybir.dt.float32
BF16 = mybir.dt.bfloat16
AF = mybir.ActivationFunctionType
OP = mybir.AluOpType
AX = mybir.AxisListType

EPS = 1e-8
EPS2 = 1e-16
B, S, D, H, L, NCLS = 4, 256, 300, 256, 10, 22
GH = 4 * H  # 1024 gates
NCHUNK = 8  # 1024 / 128
AGG_IN = 62
DX = D + 1    # x rows + ones row (bias)
AGX = AGG_IN + 1
NEG_BIG = -3.0e38

TRACE = False
DEBUG_OUTS = False

# gate chunk order in PSUM columns: i0 i1 f0 f1 o0 o1 g0 g1
# host permutes weight/bias gate blocks accordingly (torch i f g o -> i f o g)
# and scales the g-gate rows by 2 (tanh(x) = 2*sigmoid(2x) - 1).

# max-attentive q-loop routing by q-pair: D = fused mul+max on DVE,
# A / P = products on Act / Pool into 4 accumulator lanes, folded by one
# merged [128, 4S] DVE max per pair.
XACC_PAT = ('D', 'A', 'P', 'A', 'P')


class PatchedTC(tile.TileContext):
    """This walrus build rejects instructions carrying more than MAX_WAITS sync
    waits. Tile freely attaches many (one per outstanding producer proc).
    After scheduling, split the excess onto same-engine NOP carriers placed
    immediately before the overloaded instruction."""


MAX_WAITS = 1


def _split_waits(nc, maxw=None):
    if maxw is None:
        maxw = MAX_WAITS
    for f in nc.m.functions:
        for blk in f.blocks:
            insts = blk.instructions  # live list
            out = []
            for inst in insts:
                si = getattr(inst, 'sync_info', None)
                waits = list(si.on_wait) if si is not None else []
                if len(waits) > maxw:
                    excess = waits[:-maxw]
                    for w0 in range(0, len(excess), maxw):
                        nop = _make_nop(nc, inst.engine)
                        nop.sync_info = mybir.SyncInfo(
                            on_wait=excess[w0:w0 + maxw], on_update=[])
                        out.append(nop)
                    inst.sync_info = mybir.SyncInfo(
                        on_wait=waits[-maxw:], on_update=list(si.on_update))
                out.append(inst)
            if len(out) != len(insts):
                insts.clear()
                insts.extend(out)


def _make_nop(nc, engine):
    bi = nc.engines[engine].nop(nofuse=True)
    inst = bi.ins
    cur = nc.cur_bb.bb.instructions
    assert cur and cur[-1].name == inst.name
    cur.pop()
    return inst


# ----------------------------------------------------------------------------
# launch 1 program
# ----------------------------------------------------------------------------

def build_launch1():
    nc = bass.Bass()

    dr = {}
    dr['xAB'] = nc.dram_tensor('xAB', [DX, S, 2], BF16, kind='ExternalInput')
    for g in range(2):  # 0=fw 1=bw
        dr[f'ctx_WihT_{g}'] = nc.dram_tensor(f'ctx_WihT_{g}', [DX, GH], BF16, kind='ExternalInput')
        dr[f'ctx_WhhT_{g}'] = nc.dram_tensor(f'ctx_WhhT_{g}', [H, GH], BF16, kind='ExternalInput')
        dr[f'agg_WihT_{g}'] = nc.dram_tensor(f'agg_WihT_{g}', [AGX, GH], BF16, kind='ExternalInput')
        dr[f'agg_WhhT_{g}'] = nc.dram_tensor(f'agg_WhhT_{g}', [H, GH], BF16, kind='ExternalInput')
    # w*w, padded to 32 cols per perspective: tile a = [w3 w4 w5 w6], tile b = [w7 w8]
    dr['wsqT_a_f32'] = nc.dram_tensor('wsqT_a_f32', [H, 128], F32, kind='ExternalInput')
    dr['wsqT_b_f32'] = nc.dram_tensor('wsqT_b_f32', [H, 64], F32, kind='ExternalInput')
    dr['wsqT_a_bf16'] = nc.dram_tensor('wsqT_a_bf16', [H, 128], BF16, kind='ExternalInput')
    dr['wsqT_b_bf16'] = nc.dram_tensor('wsqT_b_bf16', [H, 64], BF16, kind='ExternalInput')

    encB_dram = [nc.dram_tensor(f'encB_dram_{g}', [S, H], BF16) for g in range(2)]
    # rows staged for partition-broadcast: [2, S]: 0 rnB_cos, 1 rsumA_recip
    brow_dram = [nc.dram_tensor(f'brow_dram_{g}', [2, S], F32) for g in range(2)]
    # maxpool rnB rows (set g), one-shot broadcast
    browL_dram = [nc.dram_tensor(f'browL_dram_{g}', [L, S], F32) for g in range(2)]

    dr['agg_out'] = nc.dram_tensor('agg_out', [128, 2, 2], F32, kind='ExternalOutput')
    dr['meanA'] = nc.dram_tensor('meanA', [D], F32, kind='ExternalOutput')
    if DEBUG_OUTS:
        dr['enc_dbg'] = nc.dram_tensor('enc_dbg', [2, 128, 2, 2, S + 1], BF16, kind='ExternalOutput')
        dr['mv_dbg'] = nc.dram_tensor('mv_dbg', [AGX, S], BF16, kind='ExternalOutput')
        dr['sig0_dbg'] = nc.dram_tensor('sig0_dbg', [128, 16], F32, kind='ExternalOutput')

    with PatchedTC(nc) as tc:
        _emit_core_program(nc, tc, dr, encB_dram, brow_dram, browL_dram)
    _split_waits(nc)
    return nc


KCTX = [(0, 128), (128, 128), (256, DX - 256)]  # last tile includes ones row


def _emit_core_program(nc, tc, dr, encB_dram, brow_dram, browL_dram):
    with tc.tile_pool(name='persist', bufs=1) as persist:
        # ---------------- identities, weights, inputs ----------------
        id_bf16 = persist.tile([128, 128], BF16, tag='idb', name='idb')
        id_f32 = persist.tile([128, 128], F32, tag='idf', name='idf')
        masks.make_identity(nc, id_bf16[:])
        masks.make_identity(nc, id_f32[:])

        wih, whh = {}, {}
        for g in range(2):
            wih[g] = []
            for (k0, kn) in KCTX:
                t = persist.tile([kn, GH], BF16, tag=f'wih{g}_{k0}', name=f'wih{g}_{k0}')
                nc.sync.dma_start(t[:], dr[f'ctx_WihT_{g}'][k0:k0 + kn, :])
                wih[g].append(t)
            whh[g] = []
            for k in range(2):
                t = persist.tile([128, GH], BF16, tag=f'whh{g}_{k}', name=f'whh{g}_{k}')
                nc.sync.dma_start(t[:], dr[f'ctx_WhhT_{g}'][k * 128:(k + 1) * 128, :])
                whh[g].append(t)

        awih, awhh = {}, {}
        for g in range(2):
            t = persist.tile([AGX, GH], BF16, tag=f'awih{g}', name=f'awih{g}')
            nc.sync.dma_start(t[:], dr[f'agg_WihT_{g}'][:])
            awih[g] = t
            awhh[g] = []
            for k in range(2):
                t = persist.tile([128, GH], BF16, tag=f'awhh{g}_{k}', name=f'awhh{g}_{k}')
                nc.sync.dma_start(t[:], dr[f'agg_WhhT_{g}'][k * 128:(k + 1) * 128, :])
                awhh[g].append(t)

        # wsq_f[ab][k], wsq_b[ab][k]: fp32/bf16 w^2 tiles; ab=0 -> 128 cols, ab=1 -> 64
        wsq_f, wsq_b = {}, {}
        for ab, nch in ((0, 128), (1, 64)):
            wsq_f[ab], wsq_b[ab] = [], []
            abn = 'a' if ab == 0 else 'b'
            for k in range(2):
                t = persist.tile([128, nch], F32, tag=f'wsqf{abn}{k}', name=f'wsqf{abn}{k}')
                nc.sync.dma_start(t[:], dr[f'wsqT_{abn}_f32'][k * 128:(k + 1) * 128, :])
                wsq_f[ab].append(t)
                t = persist.tile([128, nch], BF16, tag=f'wsqb{abn}{k}', name=f'wsqb{abn}{k}')
                nc.sync.dma_start(t[:], dr[f'wsqT_{abn}_bf16'][k * 128:(k + 1) * 128, :])
                wsq_b[ab].append(t)

        ones_col = persist.tile([128, 1], F32, tag='ones', name='ones')
        nc.vector.memset(ones_col[:], 1.0)
        epsb = persist.tile([128, 1], F32, tag='epsb', name='epsb')
        nc.vector.memset(epsb[:], EPS2)

        # x tiles [kn, S, 2] bf16 (A, B stacked; ones row baked in)
        xk = []
        for (k0, kn) in KCTX:
            t = persist.tile([kn, S, 2], BF16, tag=f'x{k0}', name=f'x{k0}')
            nc.sync.dma_start(t[:], dr['xAB'][k0:k0 + kn, :, :])
            xk.append(t)

        # meanA from bf16 A columns (f32 accumulate)
        macc = persist.tile([128, 3], F32, tag='macc', name='macc')
        msc = persist.tile([128, 3], F32, tag='msc', name='msc')
        nc.vector.memset(macc[:], 0.0)
        for ki, (k0, kn) in enumerate(KCTX):
            rn = min(kn, D - k0)
            nc.vector.tensor_reduce(macc[0:rn, ki:ki + 1], xk[ki][0:rn, :, 0], axis=AX.X, op=OP.add)
        nc.scalar.activation(msc[:], macc[:], AF.Copy, scale=1.0 / S)
        for ki, (k0, kn) in enumerate(KCTX):
            rn = min(kn, D - k0)
            nc.sync.dma_start(dr['meanA'][k0:k0 + rn], msc[0:rn, ki:ki + 1])

        # ---------------- ctx scans ----------------
        # encT[g]: [128, (seq 2, half 2, col S+1)] bf16; fw: h_t at col t+1 (zero col 0),
        # bw: h_t at col t (zero col S).
        encT = {g: persist.tile([128, 2, 2, S + 1], BF16, tag=f'enc{g}', name=f'enc{g}') for g in range(2)}
        for g in range(2):
            zc = 0 if g == 0 else S
            nc.vector.memset(encT[g][:, :, :, zc], 0.0)

        def ctx_x(g, t, ps):
            # start=True only on the FIRST matmul into the bank: start marks
            # the whole 2KB zero-region pending-zero; repeating it would wipe
            # earlier chunks' accumulation.
            for c in range(NCHUNK):
                for ki in range(3):
                    nc.tensor.matmul(
                        ps[:, c * 2:(c + 1) * 2],
                        wih[g][ki][:, c * 128:(c + 1) * 128], xk[ki][:, t, :],
                        start=(c == 0 and ki == 0), stop=False, skip_group_check=True)

        _emit_scan(nc, tc, 'ctx', whh, ctx_x, encT, M=2, final_out=None,
                   dbg=dr.get('sig0_dbg'))
        if DEBUG_OUTS:
            for g in range(2):
                nc.sync.dma_start(dr['enc_dbg'][g], encT[g][:])

        # ---------------- matching ----------------
        mv63 = persist.tile([AGX, S], BF16, tag='mv63', name='mv63')
        ones_row = persist.tile([1, S], BF16, tag='onesrow', name='onesrow')
        nc.vector.memset(ones_row[:], 1.0)
        nc.sync.dma_start(mv63[AGG_IN:AGX, :], ones_row[:])  # ones row (agg bias)
        _emit_matching(nc, tc, dr, encT, encB_dram, brow_dram, browL_dram,
                       wsq_f, wsq_b, ones_col, epsb, id_bf16, id_f32, mv63)

        if DEBUG_OUTS:
            nc.sync.dma_start(dr['mv_dbg'][:], mv63[:])

        # ---------------- agg ----------------
        aencT = {g: persist.tile([128, 1, 2, S + 1], BF16, tag=f'aenc{g}', name=f'aenc{g}') for g in range(2)}
        for g in range(2):
            zc = 0 if g == 0 else S
            nc.vector.memset(aencT[g][:, :, :, zc], 0.0)

        def agg_x(g, t, ps):
            for c in range(NCHUNK):
                nc.tensor.matmul(
                    ps[:, c:c + 1], awih[g][:, c * 128:(c + 1) * 128], mv63[:, t:t + 1],
                    start=(c == 0), stop=False, skip_group_check=True)

        final_h = persist.tile([128, 2, 2], F32, tag='finalh', name='finalh')  # (group, half)
        _emit_scan(nc, tc, 'agg', awhh, agg_x, aencT, M=1, final_out=final_h)
        nc.sync.dma_start(dr['agg_out'][:], final_h[:])


def _emit_scan(nc, tc, name, whh, emit_x, encT, M, final_out, dbg=None):
    """Interleaved fw/bw scan groups.
    encT[g]: [128, (M seq, 2 half, S+1)] bf16.
    whh[g]: 2 k-tiles [128, 1024] bf16, gate chunks i0 i1 f0 f1 o0 o1 g0 g1
    (g rows pre-scaled 2x). emit_x(g, t, ps) emits the x+bias part for
    step t into psum tile ps (start=True, stop=False)."""
    with (
        tc.tile_pool(name=f'{name}_ps0', bufs=3, space='PSUM') as pp0,
        tc.tile_pool(name=f'{name}_ps1', bufs=3, space='PSUM') as pp1,
        tc.tile_pool(name=f'{name}_sb', bufs=4) as sb,
        tc.tile_pool(name=f'{name}_cs', bufs=1) as csp,
    ):
        pps = {0: pp0, 1: pp1}
        c_state, banks = {}, {}
        for g in range(2):
            c_state[g] = csp.tile([128, 2 * M], F32, tag=f'c{g}', name=f'c{g}')  # (half, m)
            nc.vector.memset(c_state[g][:], 0.0)
        for step in range(S):
            for g in range(2):
                t = step if g == 0 else S - 1 - step
                rd = t if g == 0 else t + 1
                wr = t + 1 if g == 0 else t
                ps = pps[g].tile([128, NCHUNK * M], F32, tag=f'gsum{g}', name=f'gsum{g}')
                emit_x(g, t, ps)
                n_mm = 0
                for c in range(NCHUNK):
                    for k in range(2):
                        nc.tensor.matmul(
                            ps[:, c * M:(c + 1) * M],
                            whh[g][k][:, c * 128:(c + 1) * 128],
                            encT[g][:, :, k, rd],
                            start=False, stop=(n_mm == 15), skip_group_check=True)
                        n_mm += 1
                # all 8 chunks through one sigmoid; chunks 6,7 hold sigma(2g)
                sig = sb.tile([128, NCHUNK * M], F32, tag=f'sig{g}', name=f'sig{g}')
                nc.scalar.activation(sig[:], ps[:], AF.Sigmoid)
                if dbg is not None and step == 0 and g == 0:
                    nc.sync.dma_start(dbg[:], sig[:])
                cs = c_state[g]
                t2 = sb.tile([128, 2 * M], F32, tag=f't2{g}', name=f't2{g}')
                nc.vector.tensor_tensor(t2[:], sig[:, 2 * M:4 * M], cs[:], OP.mult)
                t1 = sb.tile([128, 2 * M], F32, tag=f't1{g}', name=f't1{g}')
                nc.vector.scalar_tensor_tensor(
                    t1[:], sig[:, 6 * M:8 * M], -0.5, sig[:, 0:2 * M],
                    op0=OP.add, op1=OP.mult)
                # c = 2*t1 + t2
                nc.vector.scalar_tensor_tensor(
                    cs[:], t1[:], 2.0, t2[:], op0=OP.mult, op1=OP.add)
                th = sb.tile([128, 2 * M], F32, tag=f'th{g}', name=f'th{g}')
                nc.scalar.activation(th[:], cs[:], AF.Tanh)
                # h = sig_o * th; encT dest dims (m, h) permuted to (h, m)
                hout = encT[g][:, :, :, wr].transpose([0, 2, 1])
                nc.vector.tensor_tensor(
                    hout, sig[:, 4 * M:6 * M].rearrange('p (h m) -> p h m', h=2),
                    th[:].rearrange('p (h m) -> p h m', h=2), OP.mult)
                if final_out is not None and step == S - 1:
                    nc.vector.tensor_tensor(final_out[:, g, :], sig[:, 4 * M:6 * M],
                                            th[:], OP.mult)


def _emit_matching(nc, tc, dr, encT, encB_dram, brow_dram, browL_dram,
                   wsq_f, wsq_b, ones_col, epsb, id_bf16, id_f32, mv63):
    """A-side matching features into mv63 rows 0:62 bf16.

    rows: 0 cos-max, 1 cos-mean, 2:12 maxpool-f(w3), 12:22 maxpool-b(w4),
          22:32 attentive-f(w5), 32:42 attentive-b(w6),
          42:52 max-attentive-f(w7), 52:62 max-attentive-b(w8)
    w-set s in 0..5 lives in wsq tile s//4 at col offset 32*(s%4), 10 cols wide.
    Phases are emitted interleaved between g=0 and g=1 so the in-order
    engine queues always have independent work from the other direction.
    """
    QB = 16
    with tc.tile_pool(name='m_sb', bufs=1) as msb:
        colfeat = msb.tile([128, 2, 22], F32, tag='colfeat', name='colfeat')
        st = {g: {} for g in range(2)}
        for g in range(2):
            c0 = 1 if g == 0 else 0
            st[g]['eAT'] = [encT[g][:, 0, k, c0:c0 + S] for k in range(2)]
            st[g]['eBT'] = [encT[g][:, 1, k, c0:c0 + S] for k in range(2)]

        def phaseA(g):
            s = st[g]
            eAT, eBT = s['eAT'], s['eBT']
            # squares (Act)
            sqA = [msb.tile([128, S], F32, tag=f'sqA{g}{k}', name=f'sqA{g}{k}') for k in range(2)]
            sqB = [msb.tile([128, S], F32, tag=f'sqB{g}{k}', name=f'sqB{g}{k}') for k in range(2)]
            for k in range(2):
                nc.scalar.activation(sqA[k][:], eAT[k], AF.Square)
                nc.scalar.activation(sqB[k][:], eBT[k], AF.Square)
            s['sqA'], s['sqB'] = sqA, sqB
            # cos recip norms: 1/sqrt(sum sq + eps^2)
            rnA = msb.tile([128, 2], F32, tag=f'rnA{g}', name=f'rnA{g}')
            rnB = msb.tile([128, 2], F32, tag=f'rnB{g}', name=f'rnB{g}')
            with tc.tile_pool(name=f'mn{g}', bufs=2, space='PSUM') as mps:
                for dst, sq in ((rnA, sqA), (rnB, sqB)):
                    ps = mps.tile([128, 2], F32, tag='nsq', name='nsq')
                    for pt in range(2):
                        for k in range(2):
                            nc.tensor.matmul(ps[:, pt:pt + 1],
                                             sq[k][:, pt * 128:(pt + 1) * 128],
                                             ones_col[:], start=(k == 0), stop=(k == 1),
                                             skip_group_check=True)
                    nc.scalar.activation(dst[:], ps[:], AF.Sqrt, bias=epsb[:])
                    nc.vector.reciprocal(dst[:], dst[:])
            for pt in range(2):
                nc.sync.dma_start(brow_dram[g][0, pt * 128:(pt + 1) * 128], rnB[:, pt:pt + 1])
            s['rnA'] = rnA

        def phaseB(g):
            # att + rr + transposes + encB dma
            s = st[g]
            eAT, eBT = s['eAT'], s['eBT']
            att = [msb.tile([128, S], F32, tag=f'att{g}{pt}', name=f'att{g}{pt}') for pt in range(2)]
            rsum = msb.tile([128, 2], F32, tag=f'rsum{g}', name=f'rsum{g}')
            with (
                tc.tile_pool(name=f'ma{g}', bufs=2, space='PSUM') as mps,
                tc.tile_pool(name=f'mab{g}', bufs=1) as bcp,
            ):
                rnB_bc = bcp.tile([128, S], F32, tag='rnBbc', name='rnBbc')
                nc.sync.dma_start(rnB_bc[:], brow_dram[g][0:1, :].partition_broadcast(128)[:, 0, :])
                for pt in range(2):
                    nps = mps.tile([128, S], F32, tag='num', name='num')
                    for k in range(2):
                        nc.tensor.matmul(nps[:], eAT[k][:, pt * 128:(pt + 1) * 128],
                                         eBT[k], start=(k == 0), stop=(k == 1),
                                         skip_group_check=True)
                    nc.vector.scalar_tensor_tensor(
                        att[pt][:], nps[:], s['rnA'][:, pt:pt + 1], rnB_bc[:],
                        op0=OP.mult, op1=OP.mult, accum_out=rsum[:, pt:pt + 1])
                if g == 0:  # cos max/mean features use att_fw only
                    for pt in range(2):
                        nc.vector.tensor_reduce(colfeat[:, pt, 0:1], att[pt][:], axis=AX.X, op=OP.max)
                    nc.scalar.activation(colfeat[:, :, 1], rsum[:], AF.Copy, scale=1.0 / S)
                rr = msb.tile([128, 2], F32, tag=f'rr{g}', name=f'rr{g}')
                nc.vector.tensor_scalar_max(rr[:], rsum[:], EPS)
                nc.vector.reciprocal(rr[:], rr[:])
                for pt in range(2):
                    nc.sync.dma_start(brow_dram[g][1, pt * 128:(pt + 1) * 128], rr[:, pt:pt + 1])
            encB = [msb.tile([128, S], BF16, tag=f'encB{g}{qt}', name=f'encB{g}{qt}') for qt in range(2)]
            attT = [msb.tile([128, S], BF16, tag=f'attT{g}{qt}', name=f'attT{g}{qt}') for qt in range(2)]
            with tc.tile_pool(name=f'mt{g}', bufs=4, space='PSUM') as mps:
                for qt in range(2):
                    for hf in range(2):
                        tp = mps.tile([128, 128], BF16, tag='tpb', name='tpb')
                        nc.tensor.transpose(tp[:], eBT[hf][:, qt * 128:(qt + 1) * 128], id_bf16[:])
                        nc.scalar.copy(encB[qt][:, hf * 128:(hf + 1) * 128], tp[:])
                    for pt in range(2):
                        tpf = mps.tile([128, 128], F32, tag='tpf', name='tpf')
                        nc.tensor.transpose(tpf[:], att[pt][:, qt * 128:(qt + 1) * 128], id_f32[:])
                        nc.scalar.copy(attT[qt][:, pt * 128:(pt + 1) * 128], tpf[:])
                    nc.sync.dma_start(encB_dram[g][qt * 128:(qt + 1) * 128, :], encB[qt][:])
            s['att'], s['encB'], s['attT'] = att, encB, attT

        def phaseC(g):
            # attentive mean + norm sets + maxpool
            s = st[g]
            eAT, eBT = s['eAT'], s['eBT']
            meanT = [msb.tile([128, S], BF16, tag=f'meanT{g}{ht}', name=f'meanT{g}{ht}') for ht in range(2)]
            with (
                tc.tile_pool(name=f'mm{g}', bufs=2, space='PSUM') as mps,
                tc.tile_pool(name=f'mmb{g}', bufs=1) as bcp,
            ):
                rr_bc = bcp.tile([128, S], F32, tag='rrbc', name='rrbc')
                nc.sync.dma_start(rr_bc[:], brow_dram[g][1:2, :].partition_broadcast(128)[:, 0, :])
                for ht in range(2):
                    mp = mps.tile([128, S], F32, tag='meanps', name='meanps')
                    for qt in range(2):
                        nc.tensor.matmul(mp[:], s['encB'][qt][:, ht * 128:(ht + 1) * 128],
                                         s['attT'][qt][:],
                                         start=(qt == 0), stop=(qt == 1), skip_group_check=True)
                    nc.vector.tensor_tensor(meanT[ht][:], mp[:], rr_bc[:], OP.mult)
            s['meanT'] = meanT
            nsA = [msb.tile([128, S], F32, tag=f'nsA{g}{ab}', name=f'nsA{g}{ab}') for ab in range(2)]
            base = 32 * g
            rnA_l10 = msb.tile([128, S], F32, tag=f'rnAl10{g}', name=f'rnAl10{g}')
            rnB_l10 = msb.tile([128, S], F32, tag=f'rnBl10{g}', name=f'rnBl10{g}')
            with tc.tile_pool(name=f'mns{g}', bufs=2, space='PSUM') as mps:
                for ab, nch in ((0, 128), (1, 64)):
                    ps = mps.tile([128, S], F32, tag='nset', name='nset')
                    for k in range(2):
                        nc.tensor.matmul(ps[0:nch, :], wsq_f[ab][k][:], s['sqA'][k][:],
                                         start=(k == 0), stop=(k == 1), skip_group_check=True)
                    nc.scalar.copy(nsA[ab][0:nch, :], ps[0:nch, :])
                    if ab == 0:
                        nc.scalar.activation(rnA_l10[base:base + L, :], ps[base:base + L, :],
                                             AF.Sqrt, bias=epsb[base:base + L, :])
                        nc.vector.reciprocal(rnA_l10[base:base + L, :],
                                             rnA_l10[base:base + L, :])
                        ps2 = mps.tile([128, S], F32, tag='nset', name='nset')
                        for k in range(2):
                            nc.tensor.matmul(ps2[0:nch, :], wsq_f[ab][k][:], s['sqB'][k][:],
                                             start=(k == 0), stop=(k == 1), skip_group_check=True)
                        nc.scalar.activation(rnB_l10[base:base + L, :], ps2[base:base + L, :],
                                             AF.Sqrt, bias=epsb[base:base + L, :])
                        nc.vector.reciprocal(rnB_l10[base:base + L, :],
                                             rnB_l10[base:base + L, :])
                        nc.sync.dma_start(browL_dram[g][:], rnB_l10[base:base + L, :])
            s['nsA'] = nsA
            # maxpool
            rnA_l = msb.tile([128, 2, L], F32, tag=f'rnAl{g}', name=f'rnAl{g}')
            mp_acc = msb.tile([128, 2, L], F32, tag=f'mpacc{g}', name=f'mpacc{g}')
            scr = msb.tile([128, 2, S], F32, tag=f'mpscr{g}', name=f'mpscr{g}')
            with (
                tc.tile_pool(name=f'mp{g}', bufs=3, space='PSUM') as mps,
                tc.tile_pool(name=f'mpb{g}', bufs=1) as bcp,
                tc.tile_pool(name=f'mpw{g}', bufs=2) as wap,
            ):
                rl_all = bcp.tile([128, L, S], F32, tag='rlall', name='rlall')
                nc.sync.dma_start(rl_all[:], browL_dram[g][:, :].partition_broadcast(128))
                for pt in range(2):
                    tpf = mps.tile([128, L], F32, tag='tprn', name='tprn')
                    nc.tensor.transpose(tpf[:], rnA_l10[base:base + L, pt * 128:(pt + 1) * 128],
                                        id_f32[base:base + L, base:base + L])
                    nc.scalar.copy(rnA_l[:, pt, :], tpf[:])
                for l in range(L):
                    wa = [wap.tile([128, S], BF16, tag=f'wa{k}', name=f'wa{k}') for k in range(2)]
                    for k in range(2):
                        nc.scalar.activation(wa[k][:], eAT[k], AF.Copy,
                                             scale=wsq_f[0][k][:, base + l:base + l + 1])
                    for pt in range(2):
                        nps = mps.tile([128, S], F32, tag='mpnum', name='mpnum')
                        for k in range(2):
                            nc.tensor.matmul(nps[:], wa[k][:, pt * 128:(pt + 1) * 128], eBT[k],
                                             start=(k == 0), stop=(k == 1), skip_group_check=True)
                        nc.vector.tensor_tensor(scr[:, pt, :], nps[:], rl_all[:, l, :], OP.mult)
                    nc.vector.tensor_reduce(mp_acc[:, :, l:l + 1], scr[:], axis=AX.X, op=OP.max)
                for pt in range(2):
                    nc.vector.tensor_tensor(colfeat[:, pt, 2 + g * L:2 + (g + 1) * L],
                                            mp_acc[:, pt, :], rnA_l[:, pt, :], OP.mult)

        # max-attentive accumulators + per-block emitters (interleaved g0/g1)
        def xacc_init(g):
            s = st[g]
            s['xaccD'] = [msb.tile([128, S], BF16, tag=f'xaD{g}{pt}', name=f'xaD{g}{pt}')
                          for pt in range(2)]
            s['xaccM'] = msb.tile([128, 2, 2, S], BF16, tag=f'xaM{g}', name=f'xaM{g}')
            for pt in range(2):
                nc.vector.memset(s['xaccD'][pt][:], NEG_BIG)
            nc.vector.memset(s['xaccM'][:], NEG_BIG)

        def xacc_block(g, q0):
            s = st[g]
            att = s['att']
            vb = s['vbp'].tile([128, QB, H], BF16, tag='vbc', name='vbc')
            nc.sync.dma_start(vb[:], encB_dram[g][q0:q0 + QB, :].partition_broadcast(128))
            for qq in range(0, QB, 2):
                q = q0 + qq
                eng = XACC_PAT[(q // 2) % len(XACC_PAT)]
                if eng == 'D':
                    for dq in range(2):
                        for pt in range(2):
                            nc.vector.scalar_tensor_tensor(
                                s['xaccD'][pt][:], vb[:, qq + dq, :],
                                att[pt][:, q + dq:q + dq + 1],
                                s['xaccD'][pt][:], op0=OP.mult, op1=OP.max)
                else:
                    tq = s['tqp'].tile([128, 2, 2, S], BF16, tag='tq', name='tq')
                    for dq in range(2):
                        for pt in range(2):
                            if eng == 'A':
                                nc.scalar.activation(
                                    tq[:, dq, pt, :], vb[:, qq + dq, :], AF.Copy,
                                    scale=att[pt][:, q + dq:q + dq + 1])
                            else:
                                nc.gpsimd.tensor_scalar_mul(
                                    tq[:, dq, pt, :], vb[:, qq + dq, :],
                                    att[pt][:, q + dq:q + dq + 1])
                    nc.vector.tensor_tensor(s['xaccM'][:], tq[:], s['xaccM'][:], OP.max)

        def xacc_fini(g):
            s = st[g]
            xacc = [msb.tile([128, S], BF16, tag=f'xacc{g}{pt}', name=f'xacc{g}{pt}')
                    for pt in range(2)]
            for pt in range(2):
                nc.vector.tensor_tensor(xacc[pt][:], s['xaccM'][:, 0, pt, :],
                                        s['xaccM'][:, 1, pt, :], OP.max)
                nc.vector.tensor_tensor(xacc[pt][:], xacc[pt][:], s['xaccD'][pt][:], OP.max)
            xT_ = [msb.tile([128, S], BF16, tag=f'xT{g}{ht}', name=f'xT{g}{ht}') for ht in range(2)]
            with tc.tile_pool(name=f'mxp{g}', bufs=4, space='PSUM') as mps:
                for ht in range(2):
                    for pt in range(2):
                        tp = mps.tile([128, 128], BF16, tag='tpx', name='tpx')
                        nc.tensor.transpose(tp[:], xacc[pt][:, ht * 128:(ht + 1) * 128], id_bf16[:])
                        nc.scalar.copy(xT_[ht][:, pt * 128:(pt + 1) * 128], tp[:])
            s['xT'] = xT_

        def phaseD(g):
            # final mp_match: (meanT, set 2+g) rows 22+10g; (xT, set 4+g) rows 42+10g
            s = st[g]
            eAT = s['eAT']
            for vT, set_, row0 in ((s['meanT'], 2 + g, 22 + g * L), (s['xT'], 4 + g, 42 + g * L)):
                ab, off = divmod(set_, 4)
                off *= 32
                prod = [msb.tile([128, S], BF16, tag=f'prod{g}{row0}{k}', name=f'prod{g}{row0}{k}')
                        for k in range(2)]
                vsq = [msb.tile([128, S], F32, tag=f'vsq{g}{row0}{k}', name=f'vsq{g}{row0}{k}')
                       for k in range(2)]
                for k in range(2):
                    nc.vector.tensor_tensor(prod[k][:], eAT[k], vT[k][:], OP.mult)
                    nc.scalar.activation(vsq[k][:], vT[k][:], AF.Square)
                # stage this set's A-norms at base partition 0 (engine ops need equal bases)
                n1s = msb.tile([L, S], F32, tag=f'n1s{g}{row0}', name=f'n1s{g}{row0}')
                nc.sync.dma_start(n1s[:], s['nsA'][ab][off:off + L, :])
                feat = msb.tile([L, S], BF16, tag=f'feat{g}{row0}', name=f'feat{g}{row0}')
                with tc.tile_pool(name=f'mf{g}{row0}', bufs=2, space='PSUM') as mps:
                    nump = mps.tile([128, S], F32, tag='nump', name='nump')
                    n2p = mps.tile([128, S], F32, tag='n2p', name='n2p')
                    for k in range(2):
                        nc.tensor.matmul(nump[0:L, :], wsq_b[ab][k][:, off:off + L],
                                         prod[k][:], start=(k == 0), stop=(k == 1),
                                         skip_group_check=True)
                        nc.tensor.matmul(n2p[0:L, :], wsq_f[ab][k][:, off:off + L],
                                         vsq[k][:], start=(k == 0), stop=(k == 1),
                                         skip_group_check=True)
                    den = msb.tile([L, S], F32, tag=f'den{g}{row0}', name=f'den{g}{row0}')
                    nc.vector.tensor_tensor(den[:], n2p[0:L, :], n1s[:], OP.mult)
                    denr = msb.tile([L, S], F32, tag=f'denr{g}{row0}', name=f'denr{g}{row0}')
                    nc.scalar.activation(denr[:], den[:], AF.Sqrt, bias=epsb[0:L, :])
                    nc.vector.reciprocal(denr[:], denr[:])
                    nc.vector.tensor_tensor(feat[:], nump[0:L, :], denr[:], OP.mult)
                nc.sync.dma_start(mv63[row0:row0 + L, :], feat[:])

        # ---- interleaved emission ----
        for g in range(2):
            phaseA(g)
        for g in range(2):
            phaseB(g)
        for g in range(2):
            xacc_init(g)
        for g in range(2):
            phaseC(g)
        with (
            tc.tile_pool(name='mx0', bufs=3) as vbp0,
            tc.tile_pool(name='mxt0', bufs=4) as tqp0,
            tc.tile_pool(name='mx1', bufs=3) as vbp1,
            tc.tile_pool(name='mxt1', bufs=4) as tqp1,
        ):
            st[0]['vbp'], st[0]['tqp'] = vbp0, tqp0
            st[1]['vbp'], st[1]['tqp'] = vbp1, tqp1
            for q0 in range(0, S, QB):
                xacc_block(0, q0)
                xacc_block(1, q0)
        for g in range(2):
            xacc_fini(g)
        for g in range(2):
            phaseD(g)

        # --- transpose column features into mv63 rows 0:22
        with tc.tile_pool(name='cf_ps', bufs=2, space='PSUM') as cps:
            for pt in range(2):
                tp = cps.tile([22, 128], F32, tag='tpcf', name='tpcf')
                nc.tensor.transpose(tp[:], colfeat[:, pt, :], id_f32[:])
                nc.scalar.copy(mv63[0:22, pt * 128:(pt + 1) * 128], tp[:])


def build_launch2():
    nc = bass.Bass()
    NX = 4 * H + 2 + 2 * D  # 1626
    NT = 13                 # padded to 13 * 128 rows
    NH = 2 * H  # 512
    xT = nc.dram_tensor('xT', [128, NT, B], BF16, kind='ExternalInput')
    w1T = nc.dram_tensor('w1T', [128, NT, NH], BF16, kind='ExternalInput')
    b1 = nc.dram_tensor('b1', [128, 4], F32, kind='ExternalInput')
    w2T = nc.dram_tensor('w2T', [128, 4, NCLS], F32, kind='ExternalInput')
    b2 = nc.dram_tensor('b2', [NCLS, 1], F32, kind='ExternalInput')
    yT = nc.dram_tensor('yT', [NCLS, B], F32, kind='ExternalOutput')

    with PatchedTC(nc) as tc:
        with (
            tc.tile_pool(name='sb', bufs=1) as sb,
            tc.tile_pool(name='ps', bufs=4, space='PSUM') as pp,
        ):
            xt = sb.tile([128, NT, B], BF16, tag='x', name='x')
            nc.sync.dma_start(xt[:], xT[:])
            w1 = sb.tile([128, NT, NH], BF16, tag='w1', name='w1')
            nc.sync.dma_start(w1[:], w1T[:])
            b1t = sb.tile([128, 4], F32, tag='b1', name='b1')
            nc.sync.dma_start(b1t[:], b1[:])
            w2 = sb.tile([128, 4, NCLS], F32, tag='w2', name='w2')
            nc.sync.dma_start(w2[:], w2T[:])
            b2t = sb.tile([NCLS, 1], F32, tag='b2', name='b2')
            nc.sync.dma_start(b2t[:], b2[:])
            hT = sb.tile([128, 4, B], F32, tag='hT', name='hT')
            for c in range(4):
                ps = pp.tile([128, B], F32, tag='h', name='h')
                for i in range(NT):
                    nc.tensor.matmul(ps[:], w1[:, i, c * 128:(c + 1) * 128], xt[:, i, :],
                                     start=(i == 0), stop=(i == NT - 1),
                                     skip_group_check=True)
                nc.scalar.activation(hT[:, c, :], ps[:], AF.Tanh, bias=b1t[:, c:c + 1])
            ps = pp.tile([NCLS, B], F32, tag='y', name='y')
            for c in range(4):
                nc.tensor.matmul(ps[:], w2[:, c, :], hT[:, c, :],
                                 start=(c == 0), stop=(c == 3), skip_group_check=True)
            yt = sb.tile([NCLS, B], F32, tag='yt', name='yt')
            nc.scalar.activation(yt[:], ps[:], AF.Identity, bias=b2t[:])
            nc.sync.dma_start(yT[:], yt[:])
    _split_waits(nc)
    return nc


# ----------------------------------------------------------------------------
# host orchestration
# ----------------------------------------------------------------------------

_cache = {}


def _gate_perm():
    # torch gate order (i, f, g, o) blocks of H -> chip order (i, f, o, g),
    # and within each gate the two 128-halves stay in order.
    idx = np.arange(GH).reshape(4, H)
    return np.concatenate([idx[0], idx[1], idx[3], idx[2]])


def _prep_host(inputs):
    bf = ml_dtypes.bfloat16
    perm = _gate_perm()
    pr = {}
    for g, d in ((0, 'f'), (1, 'b')):
        for pref, nin in (('ctx', D), ('agg', AGG_IN)):
            wih = np.asarray(inputs[f'{pref}_Wih_{d}'], np.float32)[perm]  # [1024, IN]
            whh = np.asarray(inputs[f'{pref}_Whh_{d}'], np.float32)[perm]
            bb = np.asarray(inputs[f'{pref}_b_{d}'], np.float32)[perm]
            # tanh(g) = 2*sigmoid(2g) - 1: scale g-gate rows (chip rows 3H:4H) by 2
            wih[3 * H:] *= 2.0
            whh[3 * H:] *= 2.0
            bb = bb.copy()
            bb[3 * H:] *= 2.0
            wihx = np.concatenate([wih.T, bb[None, :]], axis=0)  # [IN+1, GH]
            pr[f'{pref}_WihT_{g}'] = np.ascontiguousarray(wihx).astype(bf)
            pr[f'{pref}_WhhT_{g}'] = np.ascontiguousarray(whh.T).astype(bf)
    # padded w^2 sets: 32 rows per perspective; tile a = w3..w6, tile b = w7, w8
    wsq_pad = np.zeros((6 * 32, H), np.float32)
    for i in range(6):
        wsq_pad[i * 32:i * 32 + L] = np.asarray(inputs[f'mp_w{i + 3}'], np.float32) ** 2
    pr['wsqT_a_f32'] = np.ascontiguousarray(wsq_pad[0:128].T)
    pr['wsqT_b_f32'] = np.ascontiguousarray(wsq_pad[128:192].T)
    pr['wsqT_a_bf16'] = pr['wsqT_a_f32'].astype(bf)
    pr['wsqT_b_bf16'] = pr['wsqT_b_f32'].astype(bf)
    return pr


def kernel(**inputs):
    if 'l1' not in _cache:
        _cache['l1'] = build_launch1()
        _cache['l2'] = build_launch2()
    nc1, nc2 = _cache['l1'], _cache['l2']

    bf = ml_dtypes.bfloat16
    pr = _prep_host(inputs)
    left = np.asarray(inputs['left'], np.float32)
    right = np.asarray(inputs['right'], np.float32)

    in_maps = []
    for b in range(B):
        for side in range(2):
            A = left[b] if side == 0 else right[b]
            Bx = right[b] if side == 0 else left[b]
            m = dict(pr)
            xab = np.empty((DX, S, 2), np.float32)
            xab[0:D, :, 0] = A.T
            xab[0:D, :, 1] = Bx.T
            xab[D, :, :] = 1.0
            m['xAB'] = xab.astype(bf)
            in_maps.append(m)

    res1 = run_bass_kernel_spmd(nc1, in_maps, list(range(8)), trace=TRACE)

    # assemble x [4, 1626]
    xs = []
    for b in range(B):
        rp = res1.results[2 * b]
        rh = res1.results[2 * b + 1]
        ap_f = rp['agg_out'][:, 0, :].T.reshape(-1)
        ap_b = rp['agg_out'][:, 1, :].T.reshape(-1)
        ah_f = rh['agg_out'][:, 0, :].T.reshape(-1)
        ah_b = rh['agg_out'][:, 1, :].T.reshape(-1)
        meanL = rp['meanA']
        meanR = rh['meanA']
        xs.append(np.concatenate([ap_f, ap_b, ah_f, ah_b, [0.5, 0.5], meanL, meanR]))
    x = np.stack(xs).astype(np.float32)

    NX, NT, NH = 4 * H + 2 + 2 * D, 13, 2 * H
    xp = np.zeros((NT * 128, B), np.float32)
    xp[0:NX] = x.T
    w1p = np.zeros((NT * 128, NH), np.float32)
    w1p[0:NX] = np.asarray(inputs['fc1_W'], np.float32).T
    m2 = {
        'xT': xp.reshape(128, NT, B, order='F').copy() if False else
              np.ascontiguousarray(xp.reshape(NT, 128, B).transpose(1, 0, 2)).astype(bf),
        'w1T': np.ascontiguousarray(w1p.reshape(NT, 128, NH).transpose(1, 0, 2)).astype(bf),
        'b1': np.ascontiguousarray(np.asarray(inputs['fc1_b'], np.float32).reshape(4, 128).T),
        'w2T': np.ascontiguousarray(
            np.asarray(inputs['fc2_W'], np.float32).T.reshape(4, 128, NCLS).transpose(1, 0, 2)),
        'b2': np.asarray(inputs['fc2_b'], np.float32).reshape(NCLS, 1),
    }
    res2 = run_bass_kernel_spmd(nc2, [m2], [0])
    y = res2.results[0]['yT'].T
    _cache['last_exec_ns'] = (res1.exec_time_ns, res2.exec_time_ns)
    return np.ascontiguousarray(y.astype(np.float32))
